# revision 1
# baseline (speedup 1.0000x reference)
"""Trainium2 Bass kernel for nn_ChoquetIntegralConstrained.

Computes: sigmoid((x @ w_eff) / weight_sum - thr) where w_eff is built from
(wc, wint) via the constraint transform, x is [16384, 8256] f32.

Strategy: pure data parallel over batch across 8 NeuronCores. Each core gets
2048 rows, processed as 16 tiles of [128 rows, 8256]. The dot product with the
replicated weight vector is one fused DVE tensor_tensor_reduce per tile
(out = x*w, accum_out = row-sum), which hides entirely under the HBM-bound
x DMA stream. The tiny constraint transform on the 8256 weights is done on the
host in fp32 (identical elementwise semantics to the reference).
"""

import sys

import numpy as np

sys.path.insert(0, "/opt/trn_rl_repo")

N_CRIT = 128
N_PAIRS = N_CRIT * (N_CRIT - 1) // 2  # 8128
D = N_CRIT + N_PAIRS  # 8256
BATCH = 16384
N_CORES = 8
ROWS_PER_CORE = BATCH // N_CORES  # 2048
P = 128  # SBUF partitions
TILES_PER_CORE = ROWS_PER_CORE // P  # 16
MIN_W = np.float32(1e-07)

_CACHE = {}


def _build_program():
    import concourse.tile as tile
    from concourse import bacc, mybir

    nc = bacc.Bacc(
        "TRN2",
        debug=False,
        target_bir_lowering=False,
        num_devices=N_CORES,
    )
    f32 = mybir.dt.float32
    x_d = nc.dram_tensor("x", [ROWS_PER_CORE, D], f32, kind="ExternalInput").ap()
    w_d = nc.dram_tensor("w1", [1, D], f32, kind="ExternalInput").ap()
    c_d = nc.dram_tensor("consts", [P, 2], f32, kind="ExternalInput").ap()
    y_d = nc.dram_tensor("y", [P, TILES_PER_CORE], f32, kind="ExternalOutput").ap()

    n_full = TILES_PER_CORE - 1  # 15 full tiles; last tile split in 4 chunks
    CH = D // 4  # 2064

    with tile.TileContext(nc) as tc:
        with (
            tc.tile_pool(name="xp", bufs=4) as xp,
            tc.tile_pool(name="xcp", bufs=3) as xcp,
            tc.tile_pool(name="wp", bufs=1) as wp,
            tc.tile_pool(name="pp", bufs=2, space="PSUM") as pp,
        ):
            # Weight broadcast across partitions via the (otherwise idle)
            # TensorEngine: ones[1,128] stationary x w_row[1,N] -> PSUM
            # [128,N], then ACT copies PSUM->SBUF. Never touches the DMA
            # engines that stream x. w_row borrows one x-tile slot; it is
            # released once the matmuls have read it. The w row rides the
            # sync ring first so the chain starts as early as possible.
            w_row = xp.tile([1, D], f32, tag="x_t")
            nc.sync.dma_start(out=w_row[:], in_=w_d[:])
            ones_t = wp.tile([1, P], f32)
            nc.gpsimd.memset(ones_t[:], 1.0)
            # w lives in 4 separate quarter tiles: Tile deps are
            # tile-granular, so quarter-q STTs only wait for quarter q of
            # the (fp32 quarter-rate) PE chain, not the whole thing.
            w_q0 = wp.tile([P, CH], f32)
            w_q1 = wp.tile([P, CH], f32)
            w_q2 = wp.tile([P, CH], f32)
            w_q3 = wp.tile([P, CH], f32)
            w_quarters = [w_q0, w_q1, w_q2, w_q3]
            MMCH = 512
            for q in range(4):
                for j in range(0, CH, MMCH):
                    n = min(MMCH, CH - j)
                    mm = pp.tile([P, MMCH], f32)
                    nc.tensor.matmul(
                        mm[:, 0:n],
                        ones_t[:],
                        w_row[:, q * CH + j : q * CH + j + n],
                        start=True,
                        stop=True,
                    )
                    nc.scalar.copy(w_quarters[q][:, j : j + n], mm[:, 0:n])
            c_t = wp.tile([P, 2], f32)
            nc.gpsimd.dma_start(out=c_t[:], in_=c_d[:])

            acc_t = wp.tile([P, TILES_PER_CORE], f32)
            # Per-tile-quarter accumulators, combined by one reduce at the end.
            accq_t = wp.tile([P, TILES_PER_CORE * 4], f32)
            # STT must write a full-size out; a stride-0 broadcast AP over a
            # [P, 1] dummy absorbs it without SBUF cost.
            dummy = wp.tile([P, 1], f32)

            # x DMAs alternate between the two HWDGE rings (SP and ACT).
            dma_engines = (nc.scalar, nc.sync)
            n_dma = 0

            def quarter_stt(src_ap, t, q):
                nc.vector.scalar_tensor_tensor(
                    out=dummy.broadcast_to((P, CH)),
                    in0=src_ap,
                    scalar=1.0,
                    in1=w_quarters[q][:],
                    op0=mybir.AluOpType.mult,
                    op1=mybir.AluOpType.mult,
                    accum_out=accq_t[:, 4 * t + q : 4 * t + q + 1],
                )

            for t in range(TILES_PER_CORE):
                rows = slice(t * P, (t + 1) * P)
                if t == 0 or t == TILES_PER_CORE - 1:
                    # First and last tiles arrive as 4 chunk DMAs so compute
                    # can begin before the whole tile (first: before the
                    # whole w chain; last: short tail).
                    for q in range(4):
                        x_c = xcp.tile([P, CH], f32)
                        dma_engines[n_dma % 2].dma_start(
                            out=x_c[:], in_=x_d[rows, q * CH : (q + 1) * CH]
                        )
                        n_dma += 1
                        quarter_stt(x_c[:], t, q)
                else:
                    x_t = xp.tile([P, D], f32, tag="x_t")
                    dma_engines[n_dma % 2].dma_start(out=x_t[:], in_=x_d[rows, :])
                    n_dma += 1
                    for q in range(4):
                        quarter_stt(x_t[:, q * CH : (q + 1) * CH], t, q)

            # Combine the 4 quarter partial sums of every tile.
            nc.vector.tensor_reduce(
                out=acc_t[:],
                in_=accq_t[:].rearrange("p (t q) -> p t q", q=4),
                axis=mybir.AxisListType.X,
                op=mybir.AluOpType.add,
            )

            y_t = wp.tile([P, TILES_PER_CORE], f32)
            nc.scalar.activation(
                out=y_t[:],
                in_=acc_t[:],
                func=mybir.ActivationFunctionType.Sigmoid,
                bias=c_t[:, 1:2],
                scale=c_t[:, 0:1],
            )
            nc.sync.dma_start(out=y_d[:], in_=y_t[:])

    nc.compile()
    return nc


def _get_program():
    if "nc" not in _CACHE:
        _CACHE["nc"] = _build_program()
    return _CACHE["nc"]


def _host_weight_prep(wc, wint, thr):
    """Mirror reference._constrained_weights + weight_sum in fp32 numpy."""
    wc = np.asarray(wc, dtype=np.float32)
    wint = np.asarray(wint, dtype=np.float32)
    wc_eff = np.where(wc < 0, MIN_W, wc)
    ii, jj = np.triu_indices(N_CRIT, k=1)
    lower = np.maximum(-wc_eff[:, ii], -wc_eff[:, jj])
    wint_eff = np.maximum(wint, lower)
    w_eff = np.concatenate([wc_eff, wint_eff], axis=1)  # [1, D]
    wsum = np.float32(wc_eff.sum(dtype=np.float32)) + np.float32(
        wint_eff.sum(dtype=np.float32)
    )
    inv_wsum = np.float32(1.0) / wsum
    neg_thr = -np.float32(np.asarray(thr).reshape(-1)[0])
    return w_eff, inv_wsum, neg_thr


def _make_in_maps(x, wc, wint, thr):
    x = np.ascontiguousarray(np.asarray(x, dtype=np.float32))
    w_eff, inv_wsum, neg_thr = _host_weight_prep(wc, wint, thr)
    w1 = np.ascontiguousarray(w_eff)
    consts = np.empty((P, 2), dtype=np.float32)
    consts[:, 0] = inv_wsum
    consts[:, 1] = neg_thr
    return [
        {
            "x": np.ascontiguousarray(x[c * ROWS_PER_CORE : (c + 1) * ROWS_PER_CORE]),
            "w1": w1,
            "consts": consts,
        }
        for c in range(N_CORES)
    ]


def _gather(results):
    # y core tile is [P, TILES]: y[p, t] = batch row t*128 + p within the shard
    parts = [
        np.asarray(results[c]["y"]).T.reshape(ROWS_PER_CORE) for c in range(N_CORES)
    ]
    return np.concatenate(parts).reshape(BATCH, 1).astype(np.float32)


def _run(x, wc, wint, thr, trace=False):
    from concourse import bass_utils

    nc = _get_program()
    in_maps = _make_in_maps(x, wc, wint, thr)
    res = bass_utils.run_bass_kernel_spmd(
        nc, in_maps, core_ids=list(range(N_CORES)), trace=trace
    )
    return _gather(res.results), res


def kernel(x, wc, wint, thr):
    out, _ = _run(x, wc, wint, thr, trace=False)
    return out



# revision 6
# speedup vs baseline: 1.0445x; 1.0445x over previous
"""Trainium2 Bass kernel for nn_ChoquetIntegralConstrained.

Computes: sigmoid((x @ w_eff) / weight_sum - thr) where w_eff is built from
(wc, wint) via the constraint transform, x is [16384, 8256] f32.

Strategy: pure data parallel over batch across 8 NeuronCores. Each core
streams its 2048x8256 f32 shard (67.6 MB) over both HWDGE rings (sync +
scalar) at the ~358 GB/s per-core HBM cap; that stream is the roofline.
Each 128-row tile lands as two column halves, one per ring, written into
the same SBUF tile. The dot product runs on the DVE as one
scalar_tensor_tensor (mult, row-sum accumulator) per half — 4128-column
ops amortize the ~0.6 us per-instruction overhead, keeping DVE busy-time
(~160 us) under the DMA stream (~191 us) so compute never paces the
pipeline. The last two tiles arrive as quarter/eighth chunks consumed
arrival-paced to shorten the drain. The weight row is broadcast to 128
partitions via single-pass bf16 PE matmuls against a ones vector (bf16 w
costs ~1.5e-4 output rel err, far inside tolerance); ACT copies PSUM to
SBUF fp32. The tiny constraint transform runs on the host in fp32.
"""

import sys

import numpy as np

sys.path.insert(0, "/opt/trn_rl_repo")

N_CRIT = 128
N_PAIRS = N_CRIT * (N_CRIT - 1) // 2  # 8128
D = N_CRIT + N_PAIRS  # 8256
BATCH = 16384
N_CORES = 8
ROWS_PER_CORE = BATCH // N_CORES  # 2048
P = 128  # SBUF partitions
TILES_PER_CORE = ROWS_PER_CORE // P  # 16
MIN_W = np.float32(1e-07)

HALF = D // 2  # 4128
QTR = D // 4  # 2064
EGT = D // 8  # 1032

_CACHE = {}


def _build_program():
    import concourse.tile as tile
    from concourse import bacc, mybir

    nc = bacc.Bacc(
        "TRN2",
        debug=False,
        target_bir_lowering=False,
        num_devices=N_CORES,
    )
    f32 = mybir.dt.float32
    bf16 = mybir.dt.bfloat16
    x_d = nc.dram_tensor("x", [ROWS_PER_CORE, D], f32, kind="ExternalInput").ap()
    w_d = nc.dram_tensor("w1", [1, D], bf16, kind="ExternalInput").ap()
    c_d = nc.dram_tensor("consts", [P, 2], f32, kind="ExternalInput").ap()
    y_d = nc.dram_tensor("y", [P, TILES_PER_CORE], f32, kind="ExternalOutput").ap()

    N_BODY = TILES_PER_CORE - 2  # tiles 0..13 full-size; 14 quarters; 15 eighths
    MMCH = 512

    with tile.TileContext(nc) as tc:
        with (
            tc.tile_pool(name="xp", bufs=3) as xp,
            tc.tile_pool(name="cp", bufs=4) as cp,
            tc.tile_pool(name="wp", bufs=1) as wp,
            tc.tile_pool(name="pp", bufs=2, space="PSUM") as pp,
        ):
            # --- weight broadcast: bf16 w row -> 128 partitions via PE ---
            w_row = wp.tile([1, D], bf16)
            nc.sync.dma_start(out=w_row[:], in_=w_d[:])
            ones_t = wp.tile([1, P], bf16)
            nc.gpsimd.memset(ones_t[:], 1.0)
            c_t = wp.tile([P, 2], f32)
            nc.gpsimd.dma_start(out=c_t[:], in_=c_d[:])

            w_h = [
                wp.tile([P, HALF], f32, name="w_h0"),
                wp.tile([P, HALF], f32, name="w_h1"),
            ]

            # Scalar-ring x triggers for tiles 0-1 are issued by ACT before
            # its PSUM->SBUF copies so that ring streams from t~10us instead
            # of waiting out the copy chain. Sync-ring triggers have no
            # conflicting work.
            x_tiles = {}

            def tile_dma(t):
                x_t = xp.tile([P, D], f32, tag="x_t")
                rows = slice(t * P, (t + 1) * P)
                nc.sync.dma_start(out=x_t[:, 0:HALF], in_=x_d[rows, 0:HALF])
                nc.scalar.dma_start(out=x_t[:, HALF:D], in_=x_d[rows, HALF:D])
                x_tiles[t] = x_t

            tile_dma(0)
            tile_dma(1)

            # 18 bf16 matmul chunks (9 per half); ACT copies PSUM -> SBUF.
            for h in range(2):
                off = 0
                while off < HALF:
                    n = min(MMCH, HALF - off)
                    mm = pp.tile([P, MMCH], f32)
                    nc.tensor.matmul(
                        mm[:, 0:n],
                        ones_t[:],
                        w_row[:, h * HALF + off : h * HALF + off + n],
                        start=True,
                        stop=True,
                    )
                    nc.scalar.copy(w_h[h][:, off : off + n], mm[:, 0:n])
                    off += n

            accq_b = wp.tile([P, N_BODY * 2], f32)  # body halves
            accq_t = wp.tile([P, 12], f32)  # t14 quarters + t15 eighths
            dummy = wp.tile([P, 1], f32)

            def stt(src_ap, w_ap, width, acc_ap):
                nc.vector.scalar_tensor_tensor(
                    out=dummy.broadcast_to((P, width)),
                    in0=src_ap,
                    scalar=1.0,
                    in1=w_ap,
                    op0=mybir.AluOpType.mult,
                    op1=mybir.AluOpType.mult,
                    accum_out=acc_ap,
                )

            # --- body: tiles 0..13, one STT per column half ---
            for t in range(N_BODY):
                if t >= 2:
                    tile_dma(t)
                x_t = x_tiles.pop(t)
                for h in range(2):
                    stt(
                        x_t[:, h * HALF : (h + 1) * HALF],
                        w_h[h][:],
                        HALF,
                        accq_b[:, 2 * t + h : 2 * t + h + 1],
                    )

            # --- tile 14 as quarter chunks, tile 15 as eighth chunks ---
            dma_eng = (nc.sync, nc.scalar)
            t14 = []
            for q in range(4):
                x_c = cp.tile([P, QTR], f32, tag="xc")
                dma_eng[q % 2].dma_start(
                    out=x_c[:],
                    in_=x_d[14 * P : 15 * P, q * QTR : (q + 1) * QTR],
                )
                t14.append(x_c)
            for q in range(4):
                stt(
                    t14[q][:],
                    w_h[q // 2][:, (q % 2) * QTR : (q % 2 + 1) * QTR],
                    QTR,
                    accq_t[:, q : q + 1],
                )
            t15 = []
            for e in range(8):
                x_c = cp.tile([P, QTR], f32, tag="xc")
                dma_eng[e % 2].dma_start(
                    out=x_c[:, 0:EGT],
                    in_=x_d[15 * P : 16 * P, e * EGT : (e + 1) * EGT],
                )
                t15.append(x_c)
            for e in range(8):
                h = e // 4
                lo = e * EGT - h * HALF
                stt(
                    t15[e][:, 0:EGT],
                    w_h[h][:, lo : lo + EGT],
                    EGT,
                    accq_t[:, 4 + e : 5 + e],
                )

            # --- finalize: combine partial sums, sigmoid, store ---
            acc_t = wp.tile([P, TILES_PER_CORE], f32)
            nc.vector.tensor_reduce(
                out=acc_t[:, 0:N_BODY],
                in_=accq_b[:].rearrange("p (t q) -> p t q", q=2),
                axis=mybir.AxisListType.X,
                op=mybir.AluOpType.add,
            )
            nc.vector.tensor_reduce(
                out=acc_t[:, N_BODY : N_BODY + 1],
                in_=accq_t[:, 0:4].rearrange("p (t q) -> p t q", q=4),
                axis=mybir.AxisListType.X,
                op=mybir.AluOpType.add,
            )
            nc.vector.tensor_reduce(
                out=acc_t[:, N_BODY + 1 : N_BODY + 2],
                in_=accq_t[:, 4:12].rearrange("p (t q) -> p t q", q=8),
                axis=mybir.AxisListType.X,
                op=mybir.AluOpType.add,
            )

            y_t = wp.tile([P, TILES_PER_CORE], f32)
            nc.scalar.activation(
                out=y_t[:],
                in_=acc_t[:],
                func=mybir.ActivationFunctionType.Sigmoid,
                bias=c_t[:, 1:2],
                scale=c_t[:, 0:1],
            )
            nc.sync.dma_start(out=y_d[:], in_=y_t[:])

    nc.compile()
    return nc


def _get_program():
    if "nc" not in _CACHE:
        _CACHE["nc"] = _build_program()
    return _CACHE["nc"]


def _host_weight_prep(wc, wint, thr):
    """Mirror reference._constrained_weights + weight_sum in fp32 numpy."""
    import ml_dtypes

    wc = np.asarray(wc, dtype=np.float32)
    wint = np.asarray(wint, dtype=np.float32)
    wc_eff = np.where(wc < 0, MIN_W, wc)
    ii, jj = np.triu_indices(N_CRIT, k=1)
    lower = np.maximum(-wc_eff[:, ii], -wc_eff[:, jj])
    wint_eff = np.maximum(wint, lower)
    w_eff = np.concatenate([wc_eff, wint_eff], axis=1)  # [1, D]
    wsum = np.float32(wc_eff.sum(dtype=np.float32)) + np.float32(
        wint_eff.sum(dtype=np.float32)
    )
    inv_wsum = np.float32(1.0) / wsum
    neg_thr = -np.float32(np.asarray(thr).reshape(-1)[0])
    return np.ascontiguousarray(w_eff.astype(ml_dtypes.bfloat16)), inv_wsum, neg_thr


def _make_in_maps(x, wc, wint, thr):
    x = np.ascontiguousarray(np.asarray(x, dtype=np.float32))
    w1, inv_wsum, neg_thr = _host_weight_prep(wc, wint, thr)
    consts = np.empty((P, 2), dtype=np.float32)
    consts[:, 0] = inv_wsum
    consts[:, 1] = neg_thr
    return [
        {
            "x": np.ascontiguousarray(x[c * ROWS_PER_CORE : (c + 1) * ROWS_PER_CORE]),
            "w1": w1,
            "consts": consts,
        }
        for c in range(N_CORES)
    ]


def _gather(results):
    # y core tile is [P, TILES]: y[p, t] = batch row t*128 + p within the shard
    parts = [
        np.asarray(results[c]["y"]).T.reshape(ROWS_PER_CORE) for c in range(N_CORES)
    ]
    return np.concatenate(parts).reshape(BATCH, 1).astype(np.float32)


def _run(x, wc, wint, thr, trace=False):
    from concourse import bass_utils

    nc = _get_program()
    in_maps = _make_in_maps(x, wc, wint, thr)
    res = bass_utils.run_bass_kernel_spmd(
        nc, in_maps, core_ids=list(range(N_CORES)), trace=trace
    )
    return _gather(res.results), res


def kernel(x, wc, wint, thr):
    out, _ = _run(x, wc, wint, thr, trace=False)
    return out
